# revision 8
# baseline (speedup 1.0000x reference)
"""Causal self-attention (B=2, T=2048, C=1024, NH=16) on 8 Trainium2 NeuronCores.

Sharding: core = (batch b, head-group hg): b = core//4, hg = core%4.
Each core handles batch b and 4 heads [4*hg, 4*hg+4), computing a partial
projection output (w_proj row-parallel). Host sums the 4 partials per batch
and adds the (adjusted) bias.

On-chip layout is fully transposed ("S^T formulation") so no transposes are
ever needed on device:
  - host supplies xT = x[b].T                              [C, T]
  - qT/kT produced as m-tiles of (wqkv.T @ xT + b)          [256+256, T]
  - v produced natural via lhsT = xT tiles                  [T, 4*64]
  - S^T[k,q] = kT_block.T @ qT  (per k-tile of 128)         [128, q-chunk]
  - P^T = exp(S^T * 0.125)  (no max subtraction: |S/8| < ~4 for this data)
  - O^T accumulated via lhsT = v_aug (v with a ones column -> row of
    softmax denominators d[q] for free)                     [65, q-chunk]
  - normalize by 1/d via K=1 broadcast matmul + DVE multiply -> yT
  - out_partial = yT.T @ w_proj_rows  (lhsT = yT directly)  [T, C]
Causal masking: only lower-triangle k-tiles are computed; diagonal tiles are
masked by multiplying exp outputs with precomputed 0/1 masks (on gpsimd).
All matmuls use float32r (tf32-like: 1 cycle/row, ~1e-4 relerr).
"""

import os
import numpy as np
from contextlib import ExitStack

import concourse.bass as bass
import concourse.tile as tile
from concourse import bacc, mybir
from concourse.bass_utils import run_bass_kernel_spmd

F32 = mybir.dt.float32
F32R = mybir.dt.float32r
EXP = mybir.ActivationFunctionType.Exp

B, T, C = 2, 2048, 1024
NH, HD = 16, 64
NCORES = 8
HPC = 4            # heads per core
CS = HPC * HD      # 256 channels per core (per q/k/v)
KT = T // 128      # 16 k-tiles
NJ = T // 512      # 4 q-chunks
SCALE = 1.0 / np.sqrt(HD)

_NC_CACHE = None


def _register_ntff_hook():
    """The agent image's ``antenv`` lacks ``axon_hooks``; inject it and
    register the ctypes NTFF profiling hook so trace=True yields timings."""
    try:
        import sys, types, importlib
        if "antenv.axon_hooks" in sys.modules:
            return True
        tb = importlib.import_module("trn_agent_boot.trn_boot")
        hook = tb._ntff_profile_via_ctypes("/opt/axon/libaxon_pjrt.so")
        if hook is None:
            return False
        mod = types.ModuleType("antenv.axon_hooks")
        state = {"hook": hook}
        mod.set_axon_ntff_profile_hook = lambda h: state.update(hook=h)
        mod.get_axon_ntff_profile_hook = lambda: state["hook"]
        sys.modules["antenv.axon_hooks"] = mod
        import antenv
        antenv.axon_hooks = mod
        return True
    except Exception:
        return False


def _build_nc():
    nc = bacc.Bacc("TRN2", target_bir_lowering=False, debug=False)

    xT = nc.dram_tensor("xT", [C, T], F32R, kind="ExternalInput").ap()
    wqkv = nc.dram_tensor("wqkv", [C, 3 * CS], F32R, kind="ExternalInput").ap()
    bqk = nc.dram_tensor("bqk", [128, 4], F32, kind="ExternalInput").ap()
    wproj = nc.dram_tensor("wproj", [CS, C], F32R, kind="ExternalInput").ap()
    masks = nc.dram_tensor("masks", [128, 4 * 512], F32R, kind="ExternalInput").ap()
    out = nc.dram_tensor("out", [T, C], F32, kind="ExternalOutput").ap()

    with tile.TileContext(nc) as tc:
        with ExitStack() as ctx:
            # ---- persistent sbuf ----
            pers = ctx.enter_context(tc.tile_pool(name="pers", bufs=1))
            qkT = [pers.tile([128, T], F32R, tag=f"qkT{m}", name=f"qkT{m}") for m in range(4)]
            # v_aug: [128 k-rows, head, kt, 65] ; col 64 = ones (denominator)
            v_sb = pers.tile([128, HPC, KT, 65], F32R, tag="v_sb")
            yT = [pers.tile([128, T], F32R, tag=f"yT{k}", name=f"yT{k}") for k in range(2)]
            masks_sb = pers.tile([128, 4 * 512], F32R, tag="masks_sb")
            bqk_sb = pers.tile([128, 4], F32, tag="bqk_sb")
            wproj_sb = [pers.tile([128, C], F32R, tag=f"wproj{k}", name=f"wproj{k}") for k in range(2)]
            ones_sb = pers.tile([1, 64], F32R, tag="ones_sb")

            nc.vector.memset(ones_sb[:].bitcast(F32), 1.0)
            nc.vector.memset(v_sb[:, :, :, 64].bitcast(F32), 1.0)
            nc.sync.dma_start(bqk_sb[:], bqk[:])
            nc.sync.dma_start(masks_sb[:], masks[:])
            for k in range(2):
                nc.sync.dma_start(wproj_sb[k][:], wproj[k * 128:(k + 1) * 128, :])

            # ---- phase 1: qkv projections ----
            with ExitStack() as ctx1:
                ph1 = ctx1.enter_context(tc.tile_pool(name="ph1", bufs=1))
                ps1 = ctx1.enter_context(tc.tile_pool(name="ps1", bufs=4, space="PSUM"))
                w_sb = [ph1.tile([128, 3 * CS], F32R, tag=f"w{k}", name=f"w{k}") for k in range(8)]
                xT_sb = [ph1.tile([128, T], F32R, tag=f"xT{k}", name=f"xT{k}") for k in range(8)]
                for k in range(8):
                    nc.sync.dma_start(w_sb[k][:], wqkv[k * 128:(k + 1) * 128, :])
                for k in range(8):
                    nc.sync.dma_start(xT_sb[k][:], xT[k * 128:(k + 1) * 128, :])

                # qT/kT m-tiles: m0=q(h0,h1) m1=q(h2,h3) m2=k(h0,h1) m3=k(h2,h3)
                for m in range(4):
                    for j in range(NJ):
                        pq = ps1.tile([128, 512], F32, tag="pqk")
                        for k in range(8):
                            nc.tensor.matmul(
                                pq[:],
                                w_sb[k][:, m * 128:(m + 1) * 128],
                                xT_sb[k][:, j * 512:(j + 1) * 512],
                                start=(k == 0), stop=(k == 7),
                            )
                        nc.vector.tensor_scalar_add(
                            qkT[m][:, j * 512:(j + 1) * 512], pq[:], bqk_sb[:, m:m + 1]
                        )

                # v natural: [T,256] via lhsT = xT tiles (no bias: folded on host)
                for t in range(KT):
                    pv = ps1.tile([128, 256], F32, tag="pv")
                    for k in range(8):
                        nc.tensor.matmul(
                            pv[:],
                            xT_sb[k][:, t * 128:(t + 1) * 128],
                            w_sb[k][:, 2 * CS:3 * CS],
                            start=(k == 0), stop=(k == 7),
                        )
                    nc.vector.tensor_copy(
                        v_sb[:, :, t, 0:64],
                        pv[:].rearrange("p (h d) -> p h d", h=HPC),
                    )

            # ---- phase 2: attention, one head at a time ----
            att = ctx.enter_context(tc.tile_pool(name="att", bufs=3))
            ctx2 = ctx.enter_context(ExitStack())
            ps_s = ctx2.enter_context(tc.tile_pool(name="ps_s", bufs=2, space="PSUM"))
            ps_o = ctx2.enter_context(tc.tile_pool(name="ps_o", bufs=3, space="PSUM"))
            ps_d = ctx2.enter_context(tc.tile_pool(name="ps_d", bufs=1, space="PSUM"))
            dpool = ctx.enter_context(tc.tile_pool(name="dpool", bufs=2))

            for h in range(HPC):
                qm, km = h // 2, 2 + h // 2
                po = 64 * (h % 2)
                qTh = qkT[qm][po:po + 64, :]
                kTh = qkT[km][po:po + 64, :]
                for jp in range(2):
                    js_pair = (2 * jp, 2 * jp + 1)
                    po_tiles = {}
                    for j in js_pair:
                        po_tiles[j] = ps_o.tile([65, 512], F32, tag="ps_o", name=f"ps_o_{j}")
                    for kt in range(4 * js_pair[1] + 4):
                        js = [j for j in js_pair if 4 * j + 3 >= kt]
                        W = 512 * len(js)
                        ps = ps_s.tile([128, 1024], F32, tag="ps_s")
                        for c, j in enumerate(js):
                            nc.tensor.matmul(
                                ps[:, c * 512:(c + 1) * 512],
                                kTh[:, kt * 128:(kt + 1) * 128],
                                qTh[:, j * 512:(j + 1) * 512],
                                start=True, stop=True,
                            )
                        pt = att.tile([128, 1024], F32R, tag="pt")
                        nc.scalar.activation(pt[:, :W], ps[:, :W], EXP, scale=SCALE)
                        for c, j in enumerate(js):
                            d = kt - 4 * j
                            if d >= 0:  # diagonal tile -> causal mask
                                nc.gpsimd.tensor_mul(
                                    pt[:, c * 512:(c + 1) * 512],
                                    pt[:, c * 512:(c + 1) * 512],
                                    masks_sb[:, d * 512:(d + 1) * 512],
                                )
                            nc.tensor.matmul(
                                po_tiles[j][:],
                                v_sb[:, h, kt, :],
                                pt[:, c * 512:(c + 1) * 512],
                                start=(kt == 0), stop=(kt == 4 * j + 3),
                            )
                    # normalize: y = O / d ; d is row 64 of po_tiles
                    for j in js_pair:
                        dinv = dpool.tile([1, 512], F32R, tag="dinv")
                        with nc.allow_low_precision(reason="f32r is 4-byte"):
                            nc.vector.reciprocal(dinv[:], po_tiles[j][64:65, :])
                        db = ps_d.tile([64, 512], F32, tag="ps_d")
                        nc.tensor.matmul(db[:], ones_sb[:], dinv[:], start=True, stop=True)
                        db_sb = dpool.tile([64, 512], F32R, tag="db_sb")
                        nc.vector.tensor_copy(db_sb[:], db[:])
                        nc.vector.tensor_mul(
                            yT[h // 2][po:po + 64, j * 512:(j + 1) * 512],
                            po_tiles[j][0:64, :],
                            db_sb[:],
                        )

            # ---- phase 3: projection ----
            ctx2.close()  # free attention PSUM pools
            ps_p = ctx.enter_context(tc.tile_pool(name="ps_p", bufs=4, space="PSUM"))
            for t in range(KT):
                ob = att.tile([128, C], F32, tag="ob")
                for n in range(2):
                    pp = ps_p.tile([128, 512], F32, tag="ps_p")
                    for kk in range(2):
                        nc.tensor.matmul(
                            pp[:],
                            yT[kk][:, t * 128:(t + 1) * 128],
                            wproj_sb[kk][:, n * 512:(n + 1) * 512],
                            start=(kk == 0), stop=(kk == 1),
                        )
                    nc.vector.tensor_copy(ob[:, n * 512:(n + 1) * 512], pp[:])
                nc.sync.dma_start(out[t * 128:(t + 1) * 128, :], ob[:])

    nc.compile()
    return nc


def _get_nc():
    global _NC_CACHE
    if _NC_CACHE is None:
        _NC_CACHE = _build_nc()
    return _NC_CACHE


def _make_masks():
    # mask_d[p, f] = 1 iff f >= p + 128*d   (q = 512j+f valid vs k = 128kt+p, d=kt-4j)
    p = np.arange(128)[:, None]
    f = np.arange(512)[None, :]
    cols = [(f >= p + 128 * d).astype(np.float32) for d in range(4)]
    return np.ascontiguousarray(np.concatenate(cols, axis=1))


def kernel(x, w_attn, b_attn, w_proj, b_proj, n_heads):
    x = np.asarray(x, dtype=np.float32)
    w_attn = np.asarray(w_attn, dtype=np.float32)
    b_attn = np.asarray(b_attn, dtype=np.float32)
    w_proj = np.asarray(w_proj, dtype=np.float32)
    b_proj = np.asarray(b_proj, dtype=np.float32)
    assert int(n_heads) == NH and x.shape == (B, T, C)

    masks = _make_masks()
    in_maps = []
    for core in range(NCORES):
        b, hg = core // 4, core % 4
        cs = hg * CS
        wq = w_attn[:, cs:cs + CS]
        wk = w_attn[:, C + cs:C + cs + CS]
        wv = w_attn[:, 2 * C + cs:2 * C + cs + CS]
        bq = b_attn[cs:cs + CS]
        bk = b_attn[C + cs:C + cs + CS]
        in_maps.append({
            "xT": np.ascontiguousarray(x[b].T),
            "wqkv": np.ascontiguousarray(np.concatenate([wq, wk, wv], axis=1)),
            "bqk": np.ascontiguousarray(
                np.stack([bq[:128], bq[128:], bk[:128], bk[128:]], axis=1)
            ),
            "wproj": np.ascontiguousarray(w_proj[cs:cs + CS, :]),
            "masks": masks,
        })

    nc = _get_nc()
    trace = bool(os.environ.get("BASS_TRACE")) and _register_ntff_hook()
    res = run_bass_kernel_spmd(
        nc, in_maps, core_ids=list(range(NCORES)), trace=trace,
    )
    globals()["_LAST_RESULTS"] = res

    # host gather: sum head-group partials per batch, add adjusted bias
    # (v-bias folds through attention+proj into a constant row: b_v @ w_proj)
    b_eff = (b_proj.astype(np.float64)
             + b_attn[2 * C:].astype(np.float64) @ w_proj.astype(np.float64))
    outp = np.zeros((B, T, C), dtype=np.float64)
    for core in range(NCORES):
        outp[core // 4] += res.results[core]["out"].astype(np.float64)
    outp += b_eff[None, None, :]
    return outp.astype(np.float32)


# revision 12
# speedup vs baseline: 1.0606x; 1.0606x over previous
"""Causal self-attention (B=2, T=2048, C=1024, NH=16) on 8 Trainium2 NeuronCores.

Sharding: core = (batch b, head-group hg): b = core//4, hg = core%4.
Each core handles batch b and 4 heads [4*hg, 4*hg+4), computing a partial
projection output (w_proj row-parallel). Host sums the 4 partials per batch
and adds the (adjusted) bias.

On-chip layout is fully transposed ("S^T formulation") so no transposes are
ever needed on device:
  - host supplies xT = x[b].T                              [C, T]
  - qT/kT produced as m-tiles of (wqkv.T @ xT + b)          [256+256, T]
  - v produced natural via lhsT = xT tiles                  [T, 4*64]
  - S^T[k,q] = kT_block.T @ qT  (per k-tile of 128)         [128, q-chunk]
  - P^T = exp(S^T * 0.125)  (no max subtraction: |S/8| < ~4 for this data)
  - O^T accumulated via lhsT = v_aug (v with a ones column -> row of
    softmax denominators d[q] for free)                     [65, q-chunk]
  - normalize by 1/d via K=1 broadcast matmul + DVE multiply -> yT
  - out_partial = yT.T @ w_proj_rows  (lhsT = yT directly)  [T, C]
Causal masking: only lower-triangle k-tiles are computed; diagonal tiles are
masked by multiplying exp outputs with precomputed 0/1 masks (on gpsimd).
All matmuls use float32r (tf32-like: 1 cycle/row, ~1e-4 relerr).
"""

import os
import numpy as np
from contextlib import ExitStack

import concourse.bass as bass
import concourse.tile as tile
from concourse import bacc, mybir
from concourse.bass_utils import run_bass_kernel_spmd

F32 = mybir.dt.float32
F32R = mybir.dt.float32r
EXP = mybir.ActivationFunctionType.Exp

B, T, C = 2, 2048, 1024
NH, HD = 16, 64
NCORES = 8
HPC = 4            # heads per core
CS = HPC * HD      # 256 channels per core (per q/k/v)
KT = T // 128      # 16 k-tiles
NJ = T // 512      # 4 q-chunks
SCALE = 1.0 / np.sqrt(HD)

_NC_CACHE = None


def _register_ntff_hook():
    """The agent image's ``antenv`` lacks ``axon_hooks``; inject it and
    register the ctypes NTFF profiling hook so trace=True yields timings."""
    try:
        import sys, types, importlib
        if "antenv.axon_hooks" in sys.modules:
            return True
        tb = importlib.import_module("trn_agent_boot.trn_boot")
        hook = tb._ntff_profile_via_ctypes("/opt/axon/libaxon_pjrt.so")
        if hook is None:
            return False
        mod = types.ModuleType("antenv.axon_hooks")
        state = {"hook": hook}
        mod.set_axon_ntff_profile_hook = lambda h: state.update(hook=h)
        mod.get_axon_ntff_profile_hook = lambda: state["hook"]
        sys.modules["antenv.axon_hooks"] = mod
        import antenv
        antenv.axon_hooks = mod
        return True
    except Exception:
        return False


def _build_nc():
    nc = bacc.Bacc("TRN2", target_bir_lowering=False, debug=False)

    xT = nc.dram_tensor("xT", [C, T], F32R, kind="ExternalInput").ap()
    wqkv = nc.dram_tensor("wqkv", [C, 3 * CS], F32R, kind="ExternalInput").ap()
    bqk = nc.dram_tensor("bqk", [128, 4], F32, kind="ExternalInput").ap()
    wproj = nc.dram_tensor("wproj", [CS, C], F32R, kind="ExternalInput").ap()
    masks = nc.dram_tensor("masks", [128, 128], F32R, kind="ExternalInput").ap()
    out = nc.dram_tensor("out", [T, C], F32, kind="ExternalOutput").ap()

    with tile.TileContext(nc) as tc:
        with ExitStack() as ctx:
            # ---- persistent sbuf ----
            pers = ctx.enter_context(tc.tile_pool(name="pers", bufs=1))
            qkT = [pers.tile([128, T], F32R, tag=f"qkT{m}", name=f"qkT{m}") for m in range(4)]
            # v_aug: [128 k-rows, head, kt, 65] ; col 64 = ones (denominator)
            v_sb = pers.tile([128, HPC, KT, 65], F32R, tag="v_sb")
            yT = [pers.tile([128, T], F32R, tag=f"yT{k}", name=f"yT{k}") for k in range(2)]
            masks_sb = pers.tile([128, 128], F32R, tag="masks_sb")
            bqk_sb = pers.tile([128, 4], F32, tag="bqk_sb")
            wproj_sb = [pers.tile([128, C], F32R, tag=f"wproj{k}", name=f"wproj{k}") for k in range(2)]
            ones_sb = pers.tile([65, 64], F32R, tag="ones_sb")

            nc.vector.memset(ones_sb[64:65, :].bitcast(F32), 1.0)
            nc.vector.memset(v_sb[:, :, :, 64].bitcast(F32), 1.0)
            nc.sync.dma_start(bqk_sb[:], bqk[:])
            nc.sync.dma_start(masks_sb[:], masks[:])
            for k in range(2):
                nc.sync.dma_start(wproj_sb[k][:], wproj[k * 128:(k + 1) * 128, :])

            # ---- phase 1: qkv projections ----
            with ExitStack() as ctx1:
                ph1 = ctx1.enter_context(tc.tile_pool(name="ph1", bufs=1))
                ps1 = ctx1.enter_context(tc.tile_pool(name="ps1", bufs=4, space="PSUM"))
                w_sb = [ph1.tile([128, 3 * CS], F32R, tag=f"w{k}", name=f"w{k}") for k in range(8)]
                xT_sb = [ph1.tile([128, T], F32R, tag=f"xT{k}", name=f"xT{k}") for k in range(8)]
                for k in range(8):
                    nc.sync.dma_start(w_sb[k][:], wqkv[k * 128:(k + 1) * 128, :])
                    nc.sync.dma_start(xT_sb[k][:], xT[k * 128:(k + 1) * 128, :])

                # qT/kT m-tiles: m0=q(h0,h1) m1=q(h2,h3) m2=k(h0,h1) m3=k(h2,h3)
                for m in range(4):
                    for j in range(NJ):
                        pq = ps1.tile([128, 512], F32, tag="pqk")
                        for k in range(8):
                            nc.tensor.matmul(
                                pq[:],
                                w_sb[k][:, m * 128:(m + 1) * 128],
                                xT_sb[k][:, j * 512:(j + 1) * 512],
                                start=(k == 0), stop=(k == 7),
                            )
                        nc.vector.tensor_scalar_add(
                            qkT[m][:, j * 512:(j + 1) * 512], pq[:], bqk_sb[:, m:m + 1]
                        )

                # v natural: [T,256] via lhsT = xT tiles (no bias: folded on host)
                for t in range(KT):
                    pv = ps1.tile([128, 256], F32, tag="pv")
                    for k in range(8):
                        nc.tensor.matmul(
                            pv[:],
                            xT_sb[k][:, t * 128:(t + 1) * 128],
                            w_sb[k][:, 2 * CS:3 * CS],
                            start=(k == 0), stop=(k == 7),
                        )
                    nc.vector.tensor_copy(
                        v_sb[:, :, t, 0:64],
                        pv[:].rearrange("p (h d) -> p h d", h=HPC),
                    )

            # ---- phase 2: attention, one head at a time ----
            att = ctx.enter_context(tc.tile_pool(name="att", bufs=3))
            ctx2 = ctx.enter_context(ExitStack())
            ps_s = ctx2.enter_context(tc.tile_pool(name="ps_s", bufs=2, space="PSUM"))
            ps_o = ctx2.enter_context(tc.tile_pool(name="ps_o", bufs=4, space="PSUM"))
            dpool = ctx.enter_context(tc.tile_pool(name="dpool", bufs=2))
            # O^T + denominator rows for all 16 (head, chunk) pairs
            o_cache = att.tile([65, HPC * NJ, 512], F32R, tag="o_cache", bufs=1)

            for h in range(HPC):
                qm, km = h // 2, 2 + h // 2
                po = 64 * (h % 2)
                qTh = qkT[qm][po:po + 64, :]
                kTh = qkT[km][po:po + 64, :]
                for jp in range(2):
                    js_pair = (2 * jp, 2 * jp + 1)
                    po_tiles = {}
                    for j in js_pair:
                        po_tiles[j] = ps_o.tile([65, 512], F32, tag="ps_o", name=f"ps_o_{j}")
                    for kt in range(4 * js_pair[1] + 4):
                        js = [j for j in js_pair if 4 * j + 3 >= kt]
                        W = 512 * len(js)
                        ps = ps_s.tile([128, 1024], F32, tag="ps_s")
                        for c, j in enumerate(js):
                            nc.tensor.matmul(
                                ps[:, c * 512:(c + 1) * 512],
                                kTh[:, kt * 128:(kt + 1) * 128],
                                qTh[:, j * 512:(j + 1) * 512],
                                start=True, stop=True,
                            )
                        pt = att.tile([128, 1024], F32R, tag="pt", bufs=6)
                        nc.scalar.activation(pt[:, :W], ps[:, :W], EXP, scale=SCALE)
                        for c, j in enumerate(js):
                            d = kt - 4 * j
                            if d >= 0:  # diagonal tile -> causal mask
                                c0 = c * 512
                                if d > 0:  # zero fully-masked columns
                                    nc.vector.memset(pt[:, c0:c0 + 128 * d].bitcast(F32), 0.0)
                                nc.vector.tensor_mul(
                                    pt[:, c0 + 128 * d:c0 + 128 * d + 128],
                                    pt[:, c0 + 128 * d:c0 + 128 * d + 128],
                                    masks_sb[:],
                                )
                            nc.tensor.matmul(
                                po_tiles[j][:],
                                v_sb[:, h, kt, :],
                                pt[:, c * 512:(c + 1) * 512],
                                start=(kt == 0), stop=(kt == 4 * j + 3),
                            )
                    # stash O^T (+ d row) in SBUF; normalization deferred
                    for j in js_pair:
                        nc.vector.tensor_copy(o_cache[:, h * NJ + j, :], po_tiles[j][:])

            # ---- normalization end-phase: dinv = exp(-ln d), batched ----
            ctx2.close()  # free attention PSUM pools
            ps_d = ctx.enter_context(tc.tile_pool(name="ps_d", bufs=2, space="PSUM"))
            LN = mybir.ActivationFunctionType.Ln
            d_view = o_cache[64:65, :, :].rearrange("p a b -> p (a b)")
            nc.scalar.activation(d_view, d_view, LN)      # d -> ln d (in place)
            nc.scalar.activation(d_view, d_view, EXP, scale=-1.0)  # -> 1/d
            for h in range(HPC):
                po = 64 * (h % 2)
                for j in range(NJ):
                    c = h * NJ + j
                    db = ps_d.tile([64, 512], F32, tag="ps_d")
                    nc.tensor.matmul(db[:], ones_sb[64:65, :], o_cache[64:65, c, :],
                                     start=True, stop=True)
                    db_sb = dpool.tile([64, 512], F32R, tag="db_sb")
                    nc.vector.tensor_copy(db_sb[:], db[:])
                    nc.vector.tensor_mul(
                        yT[h // 2][po:po + 64, j * 512:(j + 1) * 512],
                        o_cache[0:64, c, :],
                        db_sb[:],
                    )

            # ---- phase 3: projection ----
            ps_p = ctx.enter_context(tc.tile_pool(name="ps_p", bufs=4, space="PSUM"))
            for t in range(KT):
                ob = att.tile([128, C], F32, tag="ob")
                for n in range(2):
                    pp = ps_p.tile([128, 512], F32, tag="ps_p")
                    for kk in range(2):
                        nc.tensor.matmul(
                            pp[:],
                            yT[kk][:, t * 128:(t + 1) * 128],
                            wproj_sb[kk][:, n * 512:(n + 1) * 512],
                            start=(kk == 0), stop=(kk == 1),
                        )
                    nc.vector.tensor_copy(ob[:, n * 512:(n + 1) * 512], pp[:])
                nc.sync.dma_start(out[t * 128:(t + 1) * 128, :], ob[:])

    nc.compile()
    return nc


def _get_nc():
    global _NC_CACHE
    if _NC_CACHE is None:
        _NC_CACHE = _build_nc()
    return _NC_CACHE


def _make_masks():
    # triangle: valid iff (q - k) = f - p >= 0 within the diagonal 128-block
    p = np.arange(128)[:, None]
    f = np.arange(128)[None, :]
    return np.ascontiguousarray((f >= p).astype(np.float32))


def kernel(x, w_attn, b_attn, w_proj, b_proj, n_heads):
    x = np.asarray(x, dtype=np.float32)
    w_attn = np.asarray(w_attn, dtype=np.float32)
    b_attn = np.asarray(b_attn, dtype=np.float32)
    w_proj = np.asarray(w_proj, dtype=np.float32)
    b_proj = np.asarray(b_proj, dtype=np.float32)
    assert int(n_heads) == NH and x.shape == (B, T, C)

    masks = _make_masks()
    in_maps = []
    for core in range(NCORES):
        b, hg = core // 4, core % 4
        cs = hg * CS
        wq = w_attn[:, cs:cs + CS]
        wk = w_attn[:, C + cs:C + cs + CS]
        wv = w_attn[:, 2 * C + cs:2 * C + cs + CS]
        bq = b_attn[cs:cs + CS]
        bk = b_attn[C + cs:C + cs + CS]
        in_maps.append({
            "xT": np.ascontiguousarray(x[b].T),
            "wqkv": np.ascontiguousarray(np.concatenate([wq, wk, wv], axis=1)),
            "bqk": np.ascontiguousarray(
                np.stack([bq[:128], bq[128:], bk[:128], bk[128:]], axis=1)
            ),
            "wproj": np.ascontiguousarray(w_proj[cs:cs + CS, :]),
            "masks": masks,
        })

    nc = _get_nc()
    trace = bool(os.environ.get("BASS_TRACE")) and _register_ntff_hook()
    res = run_bass_kernel_spmd(
        nc, in_maps, core_ids=list(range(NCORES)), trace=trace,
    )
    globals()["_LAST_RESULTS"] = res

    # host gather: sum head-group partials per batch, add adjusted bias
    # (v-bias folds through attention+proj into a constant row: b_v @ w_proj)
    b_eff = (b_proj.astype(np.float64)
             + b_attn[2 * C:].astype(np.float64) @ w_proj.astype(np.float64))
    outp = np.zeros((B, T, C), dtype=np.float64)
    for core in range(NCORES):
        outp[core // 4] += res.results[core]["out"].astype(np.float64)
    outp += b_eff[None, None, :]
    return outp.astype(np.float32)
